# revision 1
# baseline (speedup 1.0000x reference)
"""CTRGC graph-conv kernel for 8 Trainium2 NeuronCores.

Computes, for x:[N,C,T,V], A:[V,V], alpha, W1..W4/b1..b4 (1x1 convs):
    xm  = x.mean(T)
    x1  = W1@xm + b1 ; x2 = W2@xm + b2          (rel channels R=16)
    x3  = W3@x + b3                              (per (t,v))
    d   = tanh(x1[:,:,:,None] - x2[:,:,None,:])  (N,R,V,V)
    adj = alpha*(W4@d + b4) + A                  (N,C,V,V)
    out[n,c,t,u] = sum_v adj[n,c,v,u] * x3[n,c,t,v]

Data-parallel over batch N across 8 cores (8 batches each). Within a core,
per batch:
  - X3VT[o,(v,t)] = W3^T-contracted x via fp32r matmuls over a v-major
    strided rhs AP (b3 added during the PSUM->SBUF copy).
  - T-mean via a binary tree of elementwise adds (DVE + GpSimd).
  - adj assembled as [o,(v,u)]; the final einsum runs as 26 block-diagonal
    matmuls per batch: 5 channels pack into K=(c5,v)=125; the lhsT is an
    XC tile [(c5,v),t] gathered from X3VT with one SBUF->SBUF DMA, the rhs
    is a [125,125] block-diagonal adjacency tile filled by per-channel
    scatter DMAs; out lands [t,(c,u)] and stores with one 4D-AP DMA per
    batch.
"""
import sys

sys.path.insert(0, "/opt/trn_rl_repo")

import numpy as np

import concourse.bass as bass
import concourse.mybir as mybir
from concourse import tile
from concourse.bass_utils import run_bass_kernel_spmd

F32 = mybir.dt.float32
F32R = mybir.dt.float32r
AF = mybir.ActivationFunctionType
ADD = mybir.AluOpType.add
SUB = mybir.AluOpType.subtract
MULT = mybir.AluOpType.mult

N, C, T, V, R = 64, 128, 128, 25, 16
NCORES = 8
NPC = N // NCORES            # batches per core
TV = T * V                   # 3200
VU = V * V                   # 625
NBF = C // 5                 # 25 full channel blocks of 5
TAILC = C - 5 * NBF          # 3 tail channels
NB = NBF + 1                 # 26 blocks


# ---------------------------------------------------------------------------
# Walrus sync-wait limits workaround: this toolchain's walrus rejects >1 sync
# wait on most instructions (and any wait on Drain). Move excess waits onto
# same-engine no-ops inserted right before the instruction; sequencers
# dispatch in order so semantics are identical.
# ---------------------------------------------------------------------------
def _fixup_waits(nc):
    for bass_bb in nc.bb_map.values():
        bb = bass_bb.bb
        out = []
        changed = False
        for inst in bb.instructions:
            si = inst.sync_info
            waits = list(si.on_wait) if (si is not None and si.on_wait) else []
            cap = 0 if inst.opcode == "Drain" else 1
            if len(waits) > cap:
                for w in waits[cap:]:
                    nop = mybir.InstNoOp(
                        name=f"I-waitfix-{nc.next_id()}",
                        engine=inst.engine,
                        ins=[],
                        outs=[],
                        sync_info=mybir.SyncInfo(on_wait=[w], on_update=[]),
                    )
                    nc.register_instruction(nop)
                    out.append(nop)
                si.on_wait = waits[:cap]
                changed = True
            out.append(inst)
        if changed:
            bb.instructions = out


_orig_tile_exit = tile.TileContext.__exit__


def _patched_tile_exit(self, exc_type, exc_value, tb):
    r = _orig_tile_exit(self, exc_type, exc_value, tb)
    if exc_type is None:
        _fixup_waits(self.nc)
    return r


def _apply_tile_patch():
    if tile.TileContext.__exit__ is not _patched_tile_exit:
        tile.TileContext.__exit__ = _patched_tile_exit


# ---------------------------------------------------------------------------
# Program builder
# ---------------------------------------------------------------------------
def _build_program():
    _apply_tile_patch()
    nc = bass.Bass()

    xin = nc.declare_dram_parameter("xin", [NPC, C, TV], F32, isOutput=False)
    w3t = nc.declare_dram_parameter("w3t", [C, C], F32, isOutput=False)
    w1t = nc.declare_dram_parameter("w1t", [C, R], F32, isOutput=False)
    w2t = nc.declare_dram_parameter("w2t", [C, R], F32, isOutput=False)
    w4t = nc.declare_dram_parameter("w4t", [R, C], F32, isOutput=False)
    a128 = nc.declare_dram_parameter("a128", [C, VU], F32, isOutput=False)
    b1c = nc.declare_dram_parameter("b1c", [R, 1], F32, isOutput=False)
    b2c = nc.declare_dram_parameter("b2c", [R, 1], F32, isOutput=False)
    b3c = nc.declare_dram_parameter("b3c", [C, 1], F32, isOutput=False)
    oup = nc.declare_dram_parameter("oup", [NPC, C, T, V], F32, isOutput=True)

    with tile.TileContext(nc) as tc:
        with (
            tc.tile_pool(name="consts", bufs=1) as pc,
            tc.tile_pool(name="pin", bufs=2) as pin,
            tc.tile_pool(name="px3", bufs=2) as px3,
            tc.tile_pool(name="padj", bufs=2) as padj,
            tc.tile_pool(name="pbd", bufs=4) as pbd,
            tc.tile_pool(name="pxc", bufs=4) as pxc,
            tc.tile_pool(name="pout", bufs=2) as pout,
            tc.tile_pool(name="psmall", bufs=2) as psm,
            tc.tile_pool(name="psA", bufs=3, space="PSUM") as psA,
            tc.tile_pool(name="psB", bufs=2, space="PSUM") as psB,
            tc.tile_pool(name="psC", bufs=2, space="PSUM") as psC,
        ):
            tw3 = pc.tile([C, C], F32R, tag="w3")
            tw1 = pc.tile([C, R], F32, tag="w1")
            tw2 = pc.tile([C, R], F32, tag="w2")
            tw4 = pc.tile([R, C], F32R, tag="w4")
            ta = pc.tile([C, VU], F32, tag="a128")
            tb1 = pc.tile([R, 1], F32, tag="b1")
            tb2 = pc.tile([R, 1], F32, tag="b2")
            tb3 = pc.tile([C, 1], F32, tag="b3")
            nc.sync.dma_start(out=tw3[:], in_=w3t[:].bitcast(F32R))
            nc.sync.dma_start(out=tw1[:], in_=w1t[:])
            nc.sync.dma_start(out=tw2[:], in_=w2t[:])
            nc.sync.dma_start(out=tw4[:], in_=w4t[:].bitcast(F32R))
            nc.sync.dma_start(out=ta[:], in_=a128[:])
            nc.sync.dma_start(out=tb1[:], in_=b1c[:])
            nc.sync.dma_start(out=tb2[:], in_=b2c[:])
            nc.sync.dma_start(out=tb3[:], in_=b3c[:])

            bd_inits = [0]  # pool slots memset so far

            for n in range(NPC):
                # -- load x[n] --------------------------------------------
                xt = pin.tile([C, TV], F32R, tag="x")
                nc.sync.dma_start(out=xt[:], in_=xin[n].bitcast(F32R))
                xf = xt[:].bitcast(F32)

                # -- X3VT[o, (v,t)] = W3 @ x + b3 -------------------------
                x3 = px3.tile([C, TV], F32, tag="x3")
                xv = xt[:].rearrange("c (t v) -> c v t", v=V)
                for g in range(7):
                    vw = 4 if g < 6 else 1
                    ncols = vw * T
                    ps = psA.tile([C, 512], F32, tag="psA")
                    nc.tensor.matmul(
                        ps[:, 0:ncols], tw3[:], xv[:, 4 * g:4 * g + vw, :],
                        start=True, stop=True,
                    )
                    dst = x3[:, 4 * g * T:4 * g * T + ncols]
                    if g % 2 == 0:
                        nc.vector.tensor_scalar(
                            out=dst, in0=ps[:, 0:ncols],
                            scalar1=tb3[:], scalar2=None, op0=ADD,
                        )
                    else:
                        nc.scalar.activation(
                            dst, ps[:, 0:ncols], AF.Identity, bias=tb3[:],
                        )

                # -- T-mean tree -> xs [C, V] ------------------------------
                st = psm.tile([C, TV // 2], F32, tag="tree")
                nc.vector.tensor_tensor(
                    out=st[:, 0:1600], in0=xf[:, 0:1600], in1=xf[:, 1600:3200], op=ADD
                )
                w = 800
                while w >= V:
                    nc.gpsimd.tensor_tensor(
                        out=st[:, 0:w], in0=st[:, 0:w], in1=st[:, w:2 * w], op=ADD
                    )
                    w //= 2
                xs = st[:, 0:V]

                # -- x1/x2 ------------------------------------------------
                p1 = psC.tile([R, V], F32, tag="x12")
                nc.tensor.matmul(p1[:], tw1[:], xs, start=True, stop=True)
                x1 = psm.tile([R, V], F32, tag="x1sb")
                nc.vector.tensor_scalar(
                    out=x1[:], in0=p1[:],
                    scalar1=1.0 / T, scalar2=tb1[:], op0=MULT, op1=ADD,
                )
                p2 = psC.tile([R, V], F32, tag="x12")
                nc.tensor.matmul(p2[:], tw2[:], xs, start=True, stop=True)
                x2 = psm.tile([R, V], F32, tag="x2sb")
                nc.vector.tensor_scalar(
                    out=x2[:], in0=p2[:],
                    scalar1=1.0 / T, scalar2=tb2[:], op0=MULT, op1=ADD,
                )

                # -- d = tanh(x1 - x2) ------------------------------------
                dsub = psm.tile([R, VU + 1], F32, tag="dsub")
                nc.vector.memset(dsub[:, VU:VU + 1], 0.0)
                nc.vector.tensor_tensor(
                    out=dsub[:, 0:VU].rearrange("p (v u) -> p v u", u=V),
                    in0=x1[:].to_broadcast([R, V, V]),
                    in1=x2[:].rearrange("p (o u) -> p o u", o=1).broadcast_to([R, V, V]),
                    op=SUB,
                )
                dt_ = psm.tile([R, VU + 1], F32R, tag="dtanh")
                nc.scalar.activation(dt_[:], dsub[:], AF.Tanh)

                # -- adj = alphaW4 @ d + (A + alpha*b4) -------------------
                adj = padj.tile([C, VU], F32, tag="adj")
                pa1 = psA.tile([C, 512], F32, tag="psA")
                nc.tensor.matmul(pa1[:], tw4[:], dt_[:, 0:512], start=True, stop=True)
                nc.vector.tensor_tensor(
                    out=adj[:, 0:512], in0=pa1[:], in1=ta[:, 0:512], op=ADD
                )
                pa2 = psA.tile([C, 512], F32, tag="psA")
                nc.tensor.matmul(pa2[:, 0:VU + 1 - 512], tw4[:], dt_[:, 512:VU + 1], start=True, stop=True)
                nc.vector.tensor_tensor(
                    out=adj[:, 512:VU], in0=pa2[:, 0:VU - 512], in1=ta[:, 512:VU], op=ADD
                )

                # -- final einsum: 26 block-diagonal matmuls --------------
                outn = pout.tile([T, C * V], F32, tag="outn")
                for B in range(NB):
                    nch = 5 if B < NBF else TAILC
                    krows = 25 * nch
                    bd = pbd.tile([125, 125], F32, tag="bd")
                    if bd_inits[0] < 4:
                        nc.vector.memset(bd[:], 0.0)
                        bd_inits[0] += 1
                    xc = pxc.tile([125, T], F32, tag="xc")
                    nc.sync.dma_start(out=xc[0:krows, :], in_=x3[5 * B:5 * B + nch, :])
                    for j in range(nch):
                        c = 5 * B + j
                        eng = nc.sync if c % 2 == 0 else nc.scalar
                        eng.dma_start(
                            out=bd[25 * j:25 * j + 25, 25 * j:25 * j + 25],
                            in_=adj[c:c + 1, :],
                        )
                    po = psB.tile([T, 125], F32, tag="fin")
                    nc.tensor.matmul(po[:], xc[:], bd[:], start=True, stop=True)
                    dst = outn[:, 125 * B:125 * B + 25 * nch]
                    if B % 2 == 0:
                        nc.scalar.copy(dst, po[:, 0:25 * nch])
                    else:
                        nc.vector.tensor_copy(dst, po[:, 0:25 * nch])

                # -- store ------------------------------------------------
                for q in range(4):
                    nc.sync.dma_start(
                        out=oup[n, 32 * q:32 * q + 32].rearrange("c t u -> t c u"),
                        in_=outn[:, 32 * q * V:(32 * q + 32) * V].rearrange(
                            "t (c u) -> t c u", u=V),
                    )

    return nc


_PROG = None


def _get_program():
    global _PROG
    if _PROG is None:
        _PROG = _build_program()
    return _PROG


def kernel(x, A, alpha, W1, b1, W2, b2, W3, b3, W4, b4):
    out, _ = _run(x, A, alpha, W1, b1, W2, b2, W3, b3, W4, b4)
    return out


def _run(x, A, alpha, W1, b1, W2, b2, W3, b3, W4, b4, trace=False):
    x = np.ascontiguousarray(np.asarray(x, dtype=np.float32))
    A = np.asarray(A, dtype=np.float32)
    alpha_v = float(np.asarray(alpha, dtype=np.float32))
    W1 = np.asarray(W1, dtype=np.float32)
    W2 = np.asarray(W2, dtype=np.float32)
    W3 = np.asarray(W3, dtype=np.float32)
    W4 = np.asarray(W4, dtype=np.float32)
    b1 = np.asarray(b1, dtype=np.float32)
    b2 = np.asarray(b2, dtype=np.float32)
    b3 = np.asarray(b3, dtype=np.float32)
    b4 = np.asarray(b4, dtype=np.float32)

    w3t = np.ascontiguousarray(W3.T)                       # [c', o]
    w1t = np.ascontiguousarray(W1.T)                       # [c', R]
    w2t = np.ascontiguousarray(W2.T)
    w4t = np.ascontiguousarray((alpha_v * W4).T)           # [r, o]
    a128 = np.ascontiguousarray(
        np.tile(A.reshape(1, VU), (C, 1)) + alpha_v * b4[:, None]
    )                                                      # [o, (v,u)]
    b1c = b1[:, None].copy()
    b2c = b2[:, None].copy()
    b3c = b3[:, None].copy()                               # [o, 1]

    nc = _get_program()
    in_maps = []
    for i in range(NCORES):
        shard = np.ascontiguousarray(
            x[i * NPC:(i + 1) * NPC].reshape(NPC, C, TV)
        )
        in_maps.append({
            "xin": shard, "w3t": w3t, "w1t": w1t, "w2t": w2t, "w4t": w4t,
            "a128": a128, "b1c": b1c, "b2c": b2c, "b3c": b3c,
        })

    res = run_bass_kernel_spmd(nc, in_maps, list(range(NCORES)), trace=trace)
    out = np.empty((N, C, T, V), dtype=np.float32)
    for i in range(NCORES):
        out[i * NPC:(i + 1) * NPC] = res.results[i]["oup"]
    return out, res



# revision 2
# speedup vs baseline: 3.2383x; 3.2383x over previous
"""CTRGC graph-conv kernel for 8 Trainium2 NeuronCores.

Computes, for x:[N,C,T,V], A:[V,V], alpha, W1..W4/b1..b4 (1x1 convs):
    xm  = x.mean(T)
    x1  = W1@xm + b1 ; x2 = W2@xm + b2          (rel channels R=16)
    x3  = W3@x + b3                              (per (t,v))
    d   = tanh(x1[:,:,:,None] - x2[:,:,None,:])  (N,R,V,V)
    adj = alpha*(W4@d + b4) + A                  (N,C,V,V)
    out[n,c,t,u] = sum_v adj[n,c,v,u] * x3[n,c,t,v]

Data-parallel over batch N across 8 cores (8 batches each). The call is
dominated by host<->device transfer over the PJRT tunnel, so the wire
format is bf16 end to end: x ships bf16, all params ship in one packed
bf16 tensor, the output ships bf16 and is upcast host-side. Within a
core, per batch:
  - X3VT[o,(v,t)] = W3^T-contracted x via bf16 matmuls over a v-major
    strided rhs AP (b3 added during the PSUM->SBUF copy, downcast bf16).
  - T-mean via a binary tree of elementwise adds (DVE + GpSimd) in f32.
  - adj assembled as [o,(v,u)] bf16; the final einsum runs as 26
    block-diagonal matmuls per batch: 5 channels pack into K=(c5,v)=125;
    the lhsT is an XC tile [(c5,v),t] gathered from X3VT with one
    SBUF->SBUF DMA, the rhs is a [125,125] block-diagonal adjacency tile
    filled by per-channel scatter DMAs; out lands [t,(c,u)] bf16 and
    stores with one 4D-AP DMA per batch.
"""
import sys

sys.path.insert(0, "/opt/trn_rl_repo")

import numpy as np
import ml_dtypes

import jax

for _k, _v in (
    ("jax_compilation_cache_dir", "/tmp/jax_comp_cache"),
    ("jax_persistent_cache_min_compile_time_secs", 0),
    ("jax_persistent_cache_min_entry_size_bytes", -1),
):
    try:
        jax.config.update(_k, _v)
    except Exception:
        pass

import concourse.bass as bass
import concourse.mybir as mybir
from concourse import tile
from concourse.bass_utils import run_bass_kernel_spmd

F32 = mybir.dt.float32
BF16 = mybir.dt.bfloat16
AF = mybir.ActivationFunctionType
ADD = mybir.AluOpType.add
SUB = mybir.AluOpType.subtract
MULT = mybir.AluOpType.mult

N, C, T, V, R = 64, 128, 128, 25, 16
NCORES = 8
NPC = N // NCORES            # batches per core
TV = T * V                   # 3200
VU = V * V                   # 625
NBF = C // 5                 # 25 full channel blocks of 5
TAILC = C - 5 * NBF          # 3 tail channels
NB = NBF + 1                 # 26 blocks

# packed-parameter column layout (all bf16, partition dim = C)
PK_W3 = 0                    # [:, 0:128]   W3.T
PK_W1 = 128                  # [:, 128:144] W1.T
PK_W2 = 144                  # [:, 144:160] W2.T
PK_W4 = 160                  # [0:16, 160:288] (alpha*W4).T
PK_A = 288                   # [:, 288:913] tile(A) + alpha*b4
PK_B3 = 913                  # [:, 913]     b3
PK_B1 = 914                  # [0:16, 914]  b1
PK_B2 = 915                  # [0:16, 915]  b2
PK_COLS = 916


# ---------------------------------------------------------------------------
# Walrus sync-wait limits workaround: this toolchain's walrus rejects >1 sync
# wait on most instructions (and any wait on Drain). Move excess waits onto
# same-engine no-ops inserted right before the instruction; sequencers
# dispatch in order so semantics are identical.
# ---------------------------------------------------------------------------
def _fixup_waits(nc):
    for bass_bb in nc.bb_map.values():
        bb = bass_bb.bb
        out = []
        changed = False
        for inst in bb.instructions:
            si = inst.sync_info
            waits = list(si.on_wait) if (si is not None and si.on_wait) else []
            cap = 0 if inst.opcode == "Drain" else 1
            if len(waits) > cap:
                for w in waits[cap:]:
                    nop = mybir.InstNoOp(
                        name=f"I-waitfix-{nc.next_id()}",
                        engine=inst.engine,
                        ins=[],
                        outs=[],
                        sync_info=mybir.SyncInfo(on_wait=[w], on_update=[]),
                    )
                    nc.register_instruction(nop)
                    out.append(nop)
                si.on_wait = waits[:cap]
                changed = True
            out.append(inst)
        if changed:
            bb.instructions = out


_orig_tile_exit = tile.TileContext.__exit__


def _patched_tile_exit(self, exc_type, exc_value, tb):
    r = _orig_tile_exit(self, exc_type, exc_value, tb)
    if exc_type is None:
        _fixup_waits(self.nc)
    return r


def _apply_tile_patch():
    if tile.TileContext.__exit__ is not _patched_tile_exit:
        tile.TileContext.__exit__ = _patched_tile_exit


# ---------------------------------------------------------------------------
# Program builder
# ---------------------------------------------------------------------------
def _build_program():
    _apply_tile_patch()
    nc = bass.Bass()

    xin = nc.declare_dram_parameter("xin", [NPC, C, TV], BF16, isOutput=False)
    pkin = nc.declare_dram_parameter("pk", [C, PK_COLS], BF16, isOutput=False)
    oup = nc.declare_dram_parameter("oup", [NPC, C, T, V], BF16, isOutput=True)

    with tile.TileContext(nc) as tc:
        with (
            tc.tile_pool(name="consts", bufs=1) as pc,
            tc.tile_pool(name="pin", bufs=2) as pin,
            tc.tile_pool(name="px3", bufs=2) as px3,
            tc.tile_pool(name="padj", bufs=2) as padj,
            tc.tile_pool(name="pbd", bufs=4) as pbd,
            tc.tile_pool(name="pxc", bufs=4) as pxc,
            tc.tile_pool(name="pout", bufs=2) as pout,
            tc.tile_pool(name="psmall", bufs=2) as psm,
            tc.tile_pool(name="psA", bufs=3, space="PSUM") as psA,
            tc.tile_pool(name="psB", bufs=2, space="PSUM") as psB,
            tc.tile_pool(name="psC", bufs=2, space="PSUM") as psC,
        ):
            tp = pc.tile([C, PK_COLS], BF16, tag="pk")
            nc.sync.dma_start(out=tp[:], in_=pkin[:])
            tw3 = tp[:, PK_W3:PK_W3 + C]
            tw1 = tp[:, PK_W1:PK_W1 + R]
            tw2 = tp[:, PK_W2:PK_W2 + R]
            tw4 = tp[0:R, PK_W4:PK_W4 + C]

            # one-time f32 copies of the small additive params
            taf = pc.tile([C, VU], F32, tag="a128f")
            tb3 = pc.tile([C, 1], F32, tag="b3f")
            tb1 = pc.tile([R, 1], F32, tag="b1f")
            tb2 = pc.tile([R, 1], F32, tag="b2f")
            nc.vector.tensor_copy(taf[:], tp[:, PK_A:PK_A + VU])
            nc.scalar.copy(tb3[:], tp[:, PK_B3:PK_B3 + 1])
            nc.gpsimd.tensor_copy(tb1[:], tp[0:R, PK_B1:PK_B1 + 1])
            nc.gpsimd.tensor_copy(tb2[:], tp[0:R, PK_B2:PK_B2 + 1])

            bd_inits = [0]  # pool slots memset so far

            for n in range(NPC):
                # -- load x[n] --------------------------------------------
                xt = pin.tile([C, TV], BF16, tag="x")
                nc.sync.dma_start(out=xt[:], in_=xin[n])
                xf = xt[:]

                # -- X3VT[o, (v,t)] = W3 @ x + b3 -------------------------
                x3 = px3.tile([C, TV], BF16, tag="x3")
                xv = xt[:].rearrange("c (t v) -> c v t", v=V)
                for g in range(7):
                    vw = 4 if g < 6 else 1
                    ncols = vw * T
                    ps = psA.tile([C, 512], F32, tag="psA")
                    nc.tensor.matmul(
                        ps[:, 0:ncols], tw3, xv[:, 4 * g:4 * g + vw, :],
                        start=True, stop=True,
                    )
                    dst = x3[:, 4 * g * T:4 * g * T + ncols]
                    if g % 2 == 0:
                        nc.vector.tensor_scalar(
                            out=dst, in0=ps[:, 0:ncols],
                            scalar1=tb3[:], scalar2=None, op0=ADD,
                        )
                    else:
                        nc.scalar.activation(
                            dst, ps[:, 0:ncols], AF.Identity, bias=tb3[:],
                        )

                # -- T-mean tree -> xs [C, V] ------------------------------
                st = psm.tile([C, TV // 2], F32, tag="tree")
                nc.vector.tensor_tensor(
                    out=st[:, 0:1600], in0=xf[:, 0:1600], in1=xf[:, 1600:3200], op=ADD
                )
                w = 800
                while w >= V:
                    nc.gpsimd.tensor_tensor(
                        out=st[:, 0:w], in0=st[:, 0:w], in1=st[:, w:2 * w], op=ADD
                    )
                    w //= 2
                xsb = psm.tile([C, V], BF16, tag="xsb")
                nc.vector.tensor_copy(xsb[:], st[:, 0:V])

                # -- x1/x2 ------------------------------------------------
                p1 = psC.tile([R, V], F32, tag="x12")
                nc.tensor.matmul(p1[:], tw1, xsb[:], start=True, stop=True)
                x1 = psm.tile([R, V], F32, tag="x1sb")
                nc.vector.tensor_scalar(
                    out=x1[:], in0=p1[:],
                    scalar1=1.0 / T, scalar2=tb1[:], op0=MULT, op1=ADD,
                )
                p2 = psC.tile([R, V], F32, tag="x12")
                nc.tensor.matmul(p2[:], tw2, xsb[:], start=True, stop=True)
                x2 = psm.tile([R, V], F32, tag="x2sb")
                nc.vector.tensor_scalar(
                    out=x2[:], in0=p2[:],
                    scalar1=1.0 / T, scalar2=tb2[:], op0=MULT, op1=ADD,
                )

                # -- d = tanh(x1 - x2) ------------------------------------
                dsub = psm.tile([R, VU + 1], F32, tag="dsub")
                nc.vector.memset(dsub[:, VU:VU + 1], 0.0)
                nc.vector.tensor_tensor(
                    out=dsub[:, 0:VU].rearrange("p (v u) -> p v u", u=V),
                    in0=x1[:].to_broadcast([R, V, V]),
                    in1=x2[:].rearrange("p (o u) -> p o u", o=1).broadcast_to([R, V, V]),
                    op=SUB,
                )
                dt_ = psm.tile([R, VU + 1], BF16, tag="dtanh")
                nc.scalar.activation(dt_[:], dsub[:], AF.Tanh)

                # -- adj = alphaW4 @ d + (A + alpha*b4) -------------------
                adj = padj.tile([C, VU], BF16, tag="adj")
                pa1 = psA.tile([C, 512], F32, tag="psA")
                nc.tensor.matmul(pa1[:], tw4, dt_[:, 0:512], start=True, stop=True)
                nc.vector.tensor_tensor(
                    out=adj[:, 0:512], in0=pa1[:], in1=taf[:, 0:512], op=ADD
                )
                pa2 = psA.tile([C, 512], F32, tag="psA")
                nc.tensor.matmul(pa2[:, 0:VU + 1 - 512], tw4, dt_[:, 512:VU + 1], start=True, stop=True)
                nc.vector.tensor_tensor(
                    out=adj[:, 512:VU], in0=pa2[:, 0:VU - 512], in1=taf[:, 512:VU], op=ADD
                )

                # -- final einsum: 26 block-diagonal matmuls --------------
                outn = pout.tile([T, C * V], BF16, tag="outn")
                for B in range(NB):
                    nch = 5 if B < NBF else TAILC
                    krows = 25 * nch
                    bd = pbd.tile([125, 125], BF16, tag="bd")
                    if bd_inits[0] < 4:
                        nc.vector.memset(bd[:], 0.0)
                        bd_inits[0] += 1
                    xc = pxc.tile([125, T], BF16, tag="xc")
                    nc.sync.dma_start(out=xc[0:krows, :], in_=x3[5 * B:5 * B + nch, :])
                    for j in range(nch):
                        c = 5 * B + j
                        eng = nc.sync if c % 2 == 0 else nc.scalar
                        eng.dma_start(
                            out=bd[25 * j:25 * j + 25, 25 * j:25 * j + 25],
                            in_=adj[c:c + 1, :],
                        )
                    po = psB.tile([T, 125], F32, tag="fin")
                    nc.tensor.matmul(po[:], xc[:], bd[:], start=True, stop=True)
                    dst = outn[:, 125 * B:125 * B + 25 * nch]
                    if B % 2 == 0:
                        nc.scalar.copy(dst, po[:, 0:25 * nch])
                    else:
                        nc.vector.tensor_copy(dst, po[:, 0:25 * nch])

                # -- store ------------------------------------------------
                for q in range(4):
                    nc.sync.dma_start(
                        out=oup[n, 32 * q:32 * q + 32].rearrange("c t u -> t c u"),
                        in_=outn[:, 32 * q * V:(32 * q + 32) * V].rearrange(
                            "t (c u) -> t c u", u=V),
                    )

    return nc


_PROG = None


def _get_program():
    global _PROG
    if _PROG is None:
        _PROG = _build_program()
    return _PROG


def kernel(x, A, alpha, W1, b1, W2, b2, W3, b3, W4, b4):
    out, _ = _run(x, A, alpha, W1, b1, W2, b2, W3, b3, W4, b4)
    return out


def _run(x, A, alpha, W1, b1, W2, b2, W3, b3, W4, b4, trace=False):
    bf16 = ml_dtypes.bfloat16
    x = np.asarray(x, dtype=np.float32)
    A = np.asarray(A, dtype=np.float32)
    alpha_v = float(np.asarray(alpha, dtype=np.float32))
    W1 = np.asarray(W1, dtype=np.float32)
    W2 = np.asarray(W2, dtype=np.float32)
    W3 = np.asarray(W3, dtype=np.float32)
    W4 = np.asarray(W4, dtype=np.float32)
    b1 = np.asarray(b1, dtype=np.float32)
    b2 = np.asarray(b2, dtype=np.float32)
    b3 = np.asarray(b3, dtype=np.float32)
    b4 = np.asarray(b4, dtype=np.float32)

    pk = np.zeros((C, PK_COLS), dtype=bf16)
    pk[:, PK_W3:PK_W3 + C] = W3.T.astype(bf16)
    pk[:, PK_W1:PK_W1 + R] = W1.T.astype(bf16)
    pk[:, PK_W2:PK_W2 + R] = W2.T.astype(bf16)
    pk[0:R, PK_W4:PK_W4 + C] = (alpha_v * W4).T.astype(bf16)
    pk[:, PK_A:PK_A + VU] = (
        np.tile(A.reshape(1, VU), (C, 1)) + alpha_v * b4[:, None]
    ).astype(bf16)
    pk[:, PK_B3] = b3.astype(bf16)
    pk[0:R, PK_B1] = b1.astype(bf16)
    pk[0:R, PK_B2] = b2.astype(bf16)

    xb = np.ascontiguousarray(x).reshape(N, C, TV).astype(bf16)

    nc = _get_program()
    in_maps = []
    for i in range(NCORES):
        in_maps.append({"xin": xb[i * NPC:(i + 1) * NPC], "pk": pk})

    res = run_bass_kernel_spmd(nc, in_maps, list(range(NCORES)), trace=trace)
    out = np.empty((N, C, T, V), dtype=np.float32)
    for i in range(NCORES):
        out[i * NPC:(i + 1) * NPC] = res.results[i]["oup"]
    return out, res
